# revision 16
# baseline (speedup 1.0000x reference)
"""TRN2 Bass kernel for nn_AttributeClassifierHeaders (dense per-head MLP).

Computes y[b, a] = sigmoid(gelu(x @ W1[a] + b1[a]) . W2[a] + b2[a]) for 40
heads, sharded 5 heads per NeuronCore across 8 cores (head-parallel: each
head's weights are independent; x is replicated).

Stage-1 runs on the PE in fp8(e4m3) with perf_mode=DoubleRow: x and W1 are
quantized host-side (scales SX/SW keep values in e4m3's normal range; the
gelu activation un-scales via its fused `scale`), the contraction runs as 8
double-chunks of 256 (two fp8 weights per PE cell => 2x bf16 throughput).
End-to-end rel err vs the fp32 reference is ~1.3e-2 (CPU-simulated and
HW-verified), inside the 2e-2 gate. Layout per 256-chunk c: slot (p, i)
holds contraction index d = c*256 + i*128 + p, identically for the
stationary W1 tile [128, 2, 128] and the moving x tile [128, 2, 512].

gelu+bias+descale fuse on ScalarE out of PSUM (bf16 out). The per-head dot
product runs on the otherwise-idle DVE (acc += ht * w2col per hid-tile m,
bf16), then a single ones-vector matmul per (head, batch-chunk) reduces the
128 partitions into PSUM -- HW-profiled, this keeps TensorE ~92% busy on
stage-1 instead of spending ~10% on M=1 stage-2 matmuls. The reduce matmul,
sigmoid (fused bias b2) and the 2 KiB output DMA are emitted one stage-1
group late so the in-order PE queue never waits on ACT/DVE. x is
SBUF-resident in fp8 as two batch halves (8 MiB total, both live); W1
streams from HBM (contiguous per-(a,m) 256 KiB DMAs, once per half).
"""
import os
import sys
from contextlib import ExitStack

import numpy as np
import ml_dtypes

for _p in ("/root/.axon_site/_ro/trn_rl_repo", "/opt/trn_rl_repo"):
    if os.path.isdir(_p) and _p not in sys.path:
        sys.path.append(_p)

import jax  # noqa: E402
from jax.sharding import Mesh, PartitionSpec, NamedSharding  # noqa: E402
from jax.experimental.shard_map import shard_map  # noqa: E402

import concourse.bacc as bacc  # noqa: E402
import concourse.tile as tile  # noqa: E402
from concourse import mybir, bass2jax  # noqa: E402

F32 = mybir.dt.float32
F8 = mybir.dt.float8e4
BF = mybir.dt.bfloat16
AF = mybir.ActivationFunctionType
DR = mybir.MatmulPerfMode.DoubleRow
ALU = mybir.AluOpType

# problem shape (hardcoded; see module docstring)
B, D, A, H = 4096, 2048, 40, 1024
NCORES = 8
APC = A // NCORES        # 5 heads per core
KT = D // 128            # 16 contraction 128-tiles
KC = KT // 2             # 8 DoubleRow 256-chunks
MT = H // 128            # 8 hid tiles
NQ = 2                   # batch halves (both resident in SBUF as fp8)
QTR = B // NQ            # 2048
NCH = QTR // 512         # 512-wide chunks per half

SX = 16.0                # x fp8 scale (|x|<~6 -> <96, e4m3 normal range)
SW = 4096.0              # W1 fp8 scale (|W1|<=0.0221 -> <=90.5)
INV = 1.0 / (SX * SW)    # descale fused into the gelu activation

E4NP = ml_dtypes.float8_e4m3   # == mybir.dt.np(float8e4): TRN variant, max 240
BFNP = ml_dtypes.bfloat16


def build_program(repeat: int = 0):
    nc = bacc.Bacc("TRN2", target_bir_lowering=False, debug=False)
    x8_d = nc.dram_tensor("x8", [NQ * KC, 128, 2, QTR], F8,
                          kind="ExternalInput").ap()
    w1_d = nc.dram_tensor("w1p", [APC, MT, 128, KT, 128], F8,
                          kind="ExternalInput").ap()
    b1_d = nc.dram_tensor("b1p", [APC, 128, MT], F32, kind="ExternalInput").ap()
    w2_d = nc.dram_tensor("w2p", [APC, 128, MT], F32, kind="ExternalInput").ap()
    b2_d = nc.dram_tensor("b2p", [1, APC], F32, kind="ExternalInput").ap()
    y_d = nc.dram_tensor("y", [APC, B], F32, kind="ExternalOutput").ap()

    with tile.TileContext(nc) as tc, ExitStack() as ctx:
        const = ctx.enter_context(tc.tile_pool(name="const", bufs=2))
        xp = ctx.enter_context(tc.tile_pool(name="xp", bufs=2))
        wp = ctx.enter_context(tc.tile_pool(name="wp", bufs=2))
        sp = ctx.enter_context(tc.tile_pool(name="sp", bufs=3))
        hp = ctx.enter_context(tc.tile_pool(name="hp", bufs=5))
        ap_ = ctx.enter_context(tc.tile_pool(name="accp", bufs=2))
        tp = ctx.enter_context(tc.tile_pool(name="tmpp", bufs=4))
        ps1 = ctx.enter_context(tc.tile_pool(name="ps1", bufs=6, space="PSUM"))
        ps2 = ctx.enter_context(tc.tile_pool(name="ps2", bufs=2, space="PSUM"))

        def body():
            b1t = const.tile([128, APC * MT], F32, tag="b1t")
            w2t = const.tile([128, APC * MT], F32, tag="w2t")
            b2t = const.tile([1, APC], F32, tag="b2t")
            ones = const.tile([128, 1], BF, tag="ones")
            nc.vector.memset(ones[:], 1.0)
            for a in range(APC):
                nc.sync.dma_start(b1t[:, a * MT:(a + 1) * MT], b1_d[a])
                nc.sync.dma_start(w2t[:, a * MT:(a + 1) * MT], w2_d[a])
            nc.sync.dma_start(b2t[:], b2_d[:])
            # x chunk tiles for both halves, all live (8 MiB fp8 total).
            # Separate tiles per 256-chunk keep the DMA->matmul dependency
            # per-chunk so the first tile's c-outer loop starts as soon as
            # chunk 0 lands (instead of waiting for the full 8 MiB).
            xh = [[xp.tile([128, 2, QTR], F8, tag=f"xc{c}", name=f"x{hf}c{c}")
                   for c in range(KC)] for hf in range(NQ)]

            def dma_x(hf, c):
                # x loads ride the otherwise-idle gpsimd trigger queue: in
                # repeat mode the next iteration's prefetch then isn't
                # serialized behind this iteration's tail on the sync queue
                nc.gpsimd.dma_start(xh[hf][c][:], x8_d[hf * KC + c])

            # first x chunk in quarters: the very first matmul needs only
            # 128 KiB of x + 64 KiB of W1 in flight, not 768 KiB
            for q in range(4):
                nc.sync.dma_start(xh[0][0][:, :, 512 * q:512 * (q + 1)],
                                  x8_d[0][:, :, 512 * q:512 * (q + 1)])
            first = True
            for hf in range(NQ):
                for a in range(APC):
                    acc = [None] * NCH
                    # per-(head, chunk) epilogues (ones-reduce matmul +
                    # sigmoid + output DMA) are emitted one stage-1 group
                    # late so the in-order PE queue never waits on the
                    # DVE accumulation that produces their input.
                    pending = []

                    def tail(n, pt, a=a, hf=hf):
                        # runs per (m, n): gelu then DVE acc += ht * w2[m]
                        m = tail.m
                        ht = hp.tile([128, 512], BF, tag="ht", name="ht")
                        nc.scalar.activation(
                            ht[:], pt[:], AF.Gelu,
                            bias=b1t[:, a * MT + m:a * MT + m + 1],
                            scale=INV)
                        w2col = w2t[:, a * MT + m:a * MT + m + 1]
                        if m == 0:
                            acc_t = ap_.tile([128, 512], BF, tag=f"acc{n}",
                                             name="acc_t")
                            acc[n] = acc_t
                            nc.vector.tensor_scalar_mul(acc_t[:], ht[:], w2col)
                        else:
                            tmp = tp.tile([128, 512], BF, tag="tmp",
                                          name="tmp")
                            nc.vector.tensor_scalar_mul(tmp[:], ht[:], w2col)
                            nc.vector.tensor_tensor(acc[n][:], acc[n][:],
                                                    tmp[:], ALU.add)
                        if m == MT - 1:
                            def epilogue(n=n, a=a, hf=hf, acc_t=acc[n]):
                                psy = ps2.tile([1, 512], F32, tag="psy",
                                               name="psy")
                                nc.tensor.matmul(psy[:], ones[:], acc_t[:],
                                                 start=True, stop=True,
                                                 skip_group_check=True)
                                # sigmoid(z) = 0.5*tanh(z/2) + 0.5: tanh
                                # shares gelu's activation-table set, so no
                                # ACT_TABLE_LOAD ping-pong (HW-profiled at
                                # 1.5us per reload); b2p is pre-halved.
                                stg = sp.tile([1, 512], F32, tag="stg",
                                              name="stg")
                                nc.scalar.activation(
                                    stg[:], psy[:], AF.Tanh,
                                    bias=b2t[0:1, a:a + 1], scale=0.5)
                                stg2 = sp.tile([1, 512], F32, tag="stg2",
                                               name="stg2")
                                nc.vector.tensor_scalar(
                                    stg2[:], stg[:], 0.5, 0.5,
                                    ALU.mult, ALU.add)
                                nc.scalar.dma_start(
                                    y_d[a:a + 1,
                                        hf * QTR + n * 512:
                                        hf * QTR + (n + 1) * 512],
                                    stg2[:])
                            pending.append(epilogue)

                    if hf == NQ - 1 and a == APC - 1:
                        # Last head runs n-outer/m-inner: each batch-chunk's
                        # epilogue fires at 25/50/75/100% of this head's
                        # compute instead of all stacking after the final
                        # matmul (saves ~10us of end-of-kernel PE drain).
                        wl = []
                        for m in range(MT):
                            wt = wp.tile([128, KT, 128], F8, tag=f"wl{m}",
                                         name=f"wl{m}")
                            nc.sync.dma_start(wt[:], w1_d[a, m])
                            wl.append(wt)
                        for n in range(NCH):
                            for m in range(MT):
                                tail.m = m
                                pt = ps1.tile([128, 512], F32, tag="ps1")
                                for c in range(KC):
                                    nc.tensor.matmul(
                                        pt[:],
                                        wl[m][:, 2 * c:2 * c + 2, :],
                                        xh[hf][c][:, :,
                                                  n * 512:(n + 1) * 512],
                                        start=(c == 0), stop=(c == KC - 1),
                                        perf_mode=DR)
                                if pending:
                                    pending.pop(0)()
                                tail(n, pt)
                        while pending:
                            pending.pop(0)()
                        continue
                    w_next = None
                    for m in range(MT):
                        tail.m = m
                        if w_next is not None:
                            w = w_next
                            w_next = None
                        else:
                            w = wp.tile([128, KT, 128], F8, tag="w")
                            if first:
                                # ks-quads: the c-outer loop starts after
                                # ~192 KiB instead of the full 768 KiB of
                                # (W tile + x chunk)
                                for q in range(4):
                                    nc.sync.dma_start(
                                        w[:, 4 * q:4 * q + 4, :],
                                        w1_d[a, m, :, 4 * q:4 * q + 4, :])
                            else:
                                nc.sync.dma_start(w[:], w1_d[a, m])
                        if first and m == 0:
                            # pipeline the rest of half 0: two chunks in
                            # halves, then the m=1 W tile (so it lands well
                            # before the first tile's compute ends), then
                            # the remaining chunks in halves
                            def xhalves(c):
                                for i2 in range(2):
                                    nc.sync.dma_start(
                                        xh[0][c][:, :,
                                                 1024 * i2:1024 * (i2 + 1)],
                                        x8_d[c][:, :,
                                                1024 * i2:1024 * (i2 + 1)])
                            for c in range(1, 3):
                                xhalves(c)
                            w_next = wp.tile([128, KT, 128], F8, tag="w",
                                             name="w_next")
                            nc.sync.dma_start(w_next[:], w1_d[a, 1])
                            for c in range(3, KC):
                                xhalves(c)
                        if hf == 0 and a == 0 and m == 4:
                            for c in range(KC):
                                dma_x(1, c)
                        # first three tiles run c-outermost: tiles after the
                        # first need ALL x chunks in pair mode, so they'd
                        # stall until the full 4 MiB half lands; c-outer
                        # rides the chunk-arrival wave instead.
                        kouter = first or (hf == 0 and a == 0 and m <= 2)
                        first = False
                        if kouter:
                            # c-outermost so each x chunk is consumed as it
                            # arrives; uses NCH psum banks at once.
                            pts = []
                            for n in range(NCH):
                                pt_n = ps1.tile([128, 512], F32, tag="ps1",
                                                name=f"pt{n}")
                                pts.append(pt_n)
                            for c in range(KC):
                                for n in range(NCH):
                                    nc.tensor.matmul(
                                        pts[n][:],
                                        w[:, 2 * c:2 * c + 2, :],
                                        xh[hf][c][:, :, n * 512:(n + 1) * 512],
                                        start=(c == 0), stop=(c == KC - 1),
                                        perf_mode=DR)
                            while pending:
                                pending.pop(0)()
                            for n in range(NCH):
                                tail(n, pts[n])
                        else:
                            # n-chunks in pairs, c outermost within the
                            # pair: consecutive matmuls share the stationary
                            # W1 slice (one LDWEIGHTS per 2 matmuls) and the
                            # pair boundary staggers PSUM-bank turnover.
                            for g in range(NCH // 2):
                                pts = [ps1.tile([128, 512], F32, tag="ps1",
                                                name=f"pt{g}{j}")
                                       for j in range(2)]
                                for c in range(KC):
                                    for j in range(2):
                                        n = 2 * g + j
                                        nc.tensor.matmul(
                                            pts[j][:],
                                            w[:, 2 * c:2 * c + 2, :],
                                            xh[hf][c][:, :,
                                                      n * 512:(n + 1) * 512],
                                            start=(c == 0), stop=(c == KC - 1),
                                            perf_mode=DR)
                                if pending:
                                    pending.pop(0)()
                                for j in range(2):
                                    tail(2 * g + j, pts[j])
                    while pending:
                        pending.pop(0)()

        if repeat and repeat > 1:
            with tc.For_i(0, repeat, 1):
                body()
        else:
            body()
    nc.compile()
    return nc


class _Runner:
    """jit-once PJRT runner for a prebuilt Bass program (8-core SPMD)."""

    def __init__(self, nc, n_cores):
        bass2jax.install_neuronx_cc_hook()
        self.nc = nc
        self.n_cores = n_cores
        in_names, out_names, out_avals, zero_outs = [], [], [], []
        for alloc in nc.m.functions[0].allocations:
            if not isinstance(alloc, mybir.MemoryLocationSet):
                continue
            name = alloc.memorylocations[0].name
            if alloc.kind == "ExternalInput":
                in_names.append(name)
            elif alloc.kind == "ExternalOutput":
                shape = tuple(alloc.tensor_shape)
                dtype = mybir.dt.np(alloc.dtype)
                out_names.append(name)
                out_avals.append(jax.core.ShapedArray(shape, dtype))
                zero_outs.append(np.zeros(shape, dtype))
        partition_name = (nc.partition_id_tensor.name
                          if nc.partition_id_tensor else None)
        if partition_name is not None and partition_name in in_names:
            in_names.remove(partition_name)
        self.in_names = in_names
        self.out_names = out_names
        self.zero_outs = zero_outs
        n_params = len(in_names)
        n_outs = len(out_avals)
        all_in_names = list(in_names) + list(out_names)
        if partition_name is not None:
            all_in_names.append(partition_name)
        donate = tuple(range(n_params, n_params + n_outs))

        def _body(*args):
            operands = list(args)
            if partition_name is not None:
                operands.append(bass2jax.partition_id_tensor())
            outs = bass2jax._bass_exec_p.bind(
                *operands,
                out_avals=tuple(out_avals),
                in_names=tuple(all_in_names),
                out_names=tuple(out_names),
                lowering_input_output_aliases=(),
                sim_require_finite=True,
                sim_require_nnan=True,
                nc=nc,
            )
            return tuple(outs)

        devices = jax.devices()[:n_cores]
        assert len(devices) == n_cores, f"need {n_cores} neuron cores"
        self.mesh = Mesh(np.asarray(devices), ("core",))
        in_specs = (PartitionSpec("core"),) * (n_params + n_outs)
        out_specs = (PartitionSpec("core"),) * n_outs
        self.fn = jax.jit(
            shard_map(_body, mesh=self.mesh, in_specs=in_specs,
                      out_specs=out_specs, check_rep=False),
            donate_argnums=donate, keep_unused=True,
        )
        self._dev_inputs = None

    def put_inputs(self, in_maps):
        sharding = NamedSharding(self.mesh, PartitionSpec("core"))
        self._dev_inputs = [
            jax.device_put(
                np.concatenate([np.asarray(m[name]) for m in in_maps], axis=0),
                sharding)
            for name in self.in_names
        ]

    def run(self):
        sharding = NamedSharding(self.mesh, PartitionSpec("core"))
        zouts = [jax.device_put(np.concatenate([z] * self.n_cores, axis=0),
                                sharding) for z in self.zero_outs]
        outs = self.fn(*self._dev_inputs, *zouts)
        jax.block_until_ready(outs)
        return outs

    def run_np(self):
        outs = self.run()
        res = []
        for c in range(self.n_cores):
            d = {}
            for i, name in enumerate(self.out_names):
                full = np.asarray(outs[i])
                per = full.shape[0] // self.n_cores
                d[name] = full[c * per:(c + 1) * per]
            res.append(d)
        return res


_CACHE = {}


def _get_runner(repeat=0):
    if repeat not in _CACHE:
        _CACHE[repeat] = _Runner(build_program(repeat), NCORES)
    return _CACHE[repeat]


def _q8(a, scale):
    return np.clip(np.asarray(a, np.float32) * scale,
                   -240.0, 240.0).astype(E4NP)


def make_in_maps(x, W1, b1, W2, b2):
    x = np.asarray(x, dtype=np.float32)
    W1 = np.asarray(W1, dtype=np.float32)
    b1 = np.asarray(b1, dtype=np.float32)
    W2 = np.asarray(W2, dtype=np.float32)
    b2 = np.asarray(b2, dtype=np.float32)
    # x8[(hf*KC+c), p, i*QTR+n] = fp8(SX * x[hf*QTR+n, c*256+i*128+p]):
    # per-chunk contiguous 512 KiB blocks matching the [128, 2, QTR] tiles
    xq = _q8(x, SX).T
    x8 = np.ascontiguousarray(
        xq.reshape(KC, 2, 128, NQ, QTR).transpose(3, 0, 2, 1, 4)
        .reshape(NQ * KC, 128, 2, QTR))
    # W1p[a, m, p, ks, c] = fp8(SW * W1[a, ks*128+p, m*128+c]) (per-(a,m)
    # contiguous 256 KiB block; DoubleRow pairs are ks slots (2c, 2c+1))
    W1p = np.ascontiguousarray(
        _q8(W1, SW).reshape(A, KT, 128, MT, 128).transpose(0, 3, 2, 1, 4)
        .reshape(A, MT, 128, KT, 128))
    b1p = np.ascontiguousarray(b1.reshape(A, MT, 128).transpose(0, 2, 1))
    W2p = np.ascontiguousarray(W2.reshape(A, MT, 128).transpose(0, 2, 1))
    b2p = np.ascontiguousarray(0.5 * b2.reshape(1, A))  # tanh-form sigmoid
    in_maps = []
    for c in range(NCORES):
        s = slice(c * APC, (c + 1) * APC)
        in_maps.append({"x8": x8, "w1p": W1p[s], "b1p": b1p[s],
                        "w2p": W2p[s], "b2p": b2p[:, s]})
    return in_maps


def kernel(x, W1, b1, W2, b2):
    in_maps = make_in_maps(x, W1, b1, W2, b2)
    r = _get_runner(0)
    r.put_inputs(in_maps)
    outs = r.run_np()
    y = np.concatenate([outs[c]["y"] for c in range(NCORES)], axis=0)
    return np.ascontiguousarray(y.T).astype(np.float32)


# revision 18
# speedup vs baseline: 1.2028x; 1.2028x over previous
"""TRN2 Bass kernel for nn_AttributeClassifierHeaders (dense per-head MLP).

Computes y[b, a] = sigmoid(gelu(x @ W1[a] + b1[a]) . W2[a] + b2[a]) for 40
heads, sharded 5 heads per NeuronCore across 8 cores (head-parallel: each
head's weights are independent; x is replicated).

Stage-1 runs on the PE in fp8(e4m3) with perf_mode=DoubleRow: x and W1 are
quantized host-side (scales SX/SW keep values in e4m3's normal range; the
gelu activation un-scales via its fused `scale`), the contraction runs as 8
double-chunks of 256 (two fp8 weights per PE cell => 2x bf16 throughput).
End-to-end rel err vs the fp32 reference is ~1.3e-2 (CPU-simulated and
HW-verified), inside the 2e-2 gate. Layout per 256-chunk c: slot (p, i)
holds contraction index d = c*256 + i*128 + p, identically for the
stationary W1 tile [128, 2, 128] and the moving x tile [128, 2, 512].

gelu+bias+descale fuse on ScalarE out of PSUM (bf16 out). The per-head dot
product runs on the otherwise-idle DVE (acc += ht * w2col per hid-tile m,
bf16), then a single ones-vector matmul per (head, batch-chunk) reduces the
128 partitions into PSUM -- HW-profiled, this keeps TensorE ~92% busy on
stage-1 instead of spending ~10% on M=1 stage-2 matmuls. The reduce matmul,
sigmoid (fused bias b2) and the 2 KiB output DMA are emitted one stage-1
group late so the in-order PE queue never waits on ACT/DVE. x is
SBUF-resident in fp8 as two batch halves (8 MiB total, both live); W1
streams from HBM (contiguous per-(a,m) 256 KiB DMAs, once per half).
"""
import os
import sys
from contextlib import ExitStack

import numpy as np
import ml_dtypes

for _p in ("/root/.axon_site/_ro/trn_rl_repo", "/opt/trn_rl_repo"):
    if os.path.isdir(_p) and _p not in sys.path:
        sys.path.append(_p)

import jax  # noqa: E402
from jax.sharding import Mesh, PartitionSpec, NamedSharding  # noqa: E402
from jax.experimental.shard_map import shard_map  # noqa: E402

import concourse.bacc as bacc  # noqa: E402
import concourse.tile as tile  # noqa: E402
from concourse import mybir, bass2jax  # noqa: E402

F32 = mybir.dt.float32
F8 = mybir.dt.float8e4
BF = mybir.dt.bfloat16
AF = mybir.ActivationFunctionType
DR = mybir.MatmulPerfMode.DoubleRow
ALU = mybir.AluOpType

# problem shape (hardcoded; see module docstring)
B, D, A, H = 4096, 2048, 40, 1024
NCORES = 8
APC = A // NCORES        # 5 heads per core
KT = D // 128            # 16 contraction 128-tiles
KC = KT // 2             # 8 DoubleRow 256-chunks
MT = H // 128            # 8 hid tiles
NQ = 2                   # batch halves (both resident in SBUF as fp8)
QTR = B // NQ            # 2048
NCH = QTR // 512         # 512-wide chunks per half

SX = 16.0                # x fp8 scale (|x|<~6 -> <96, e4m3 normal range)
SW = 4096.0              # W1 fp8 scale (|W1|<=0.0221 -> <=90.5)
INV = 1.0 / (SX * SW)    # descale fused into the gelu activation

E4NP = ml_dtypes.float8_e4m3   # == mybir.dt.np(float8e4): TRN variant, max 240
BFNP = ml_dtypes.bfloat16


def build_program(repeat: int = 0):
    nc = bacc.Bacc("TRN2", target_bir_lowering=False, debug=False)
    x8_d = nc.dram_tensor("x8", [NQ * KC, 128, 2, QTR], F8,
                          kind="ExternalInput").ap()
    w1_d = nc.dram_tensor("w1p", [APC, MT, 128, KT, 128], F8,
                          kind="ExternalInput").ap()
    b1_d = nc.dram_tensor("b1p", [APC, 128, MT], F32, kind="ExternalInput").ap()
    w2_d = nc.dram_tensor("w2p", [APC, 128, MT], F32, kind="ExternalInput").ap()
    b2_d = nc.dram_tensor("b2p", [1, APC], F32, kind="ExternalInput").ap()
    y_d = nc.dram_tensor("y", [APC, B], F32, kind="ExternalOutput").ap()

    with tile.TileContext(nc) as tc, ExitStack() as ctx:
        const = ctx.enter_context(tc.tile_pool(name="const", bufs=2))
        xp = ctx.enter_context(tc.tile_pool(name="xp", bufs=2))
        wp = ctx.enter_context(tc.tile_pool(name="wp", bufs=2))
        sp = ctx.enter_context(tc.tile_pool(name="sp", bufs=3))
        hp = ctx.enter_context(tc.tile_pool(name="hp", bufs=5))
        ap_ = ctx.enter_context(tc.tile_pool(name="accp", bufs=2))
        tp = ctx.enter_context(tc.tile_pool(name="tmpp", bufs=4))
        ps1 = ctx.enter_context(tc.tile_pool(name="ps1", bufs=6, space="PSUM"))
        ps2 = ctx.enter_context(tc.tile_pool(name="ps2", bufs=2, space="PSUM"))

        def body():
            b1t = const.tile([128, APC * MT], F32, tag="b1t")
            w2t = const.tile([128, APC * MT], F32, tag="w2t")
            b2t = const.tile([1, APC], F32, tag="b2t")
            ones = const.tile([128, 1], BF, tag="ones")
            nc.vector.memset(ones[:], 1.0)
            for a in range(APC):
                nc.sync.dma_start(b1t[:, a * MT:(a + 1) * MT], b1_d[a])
                nc.sync.dma_start(w2t[:, a * MT:(a + 1) * MT], w2_d[a])
            nc.sync.dma_start(b2t[:], b2_d[:])
            # x chunk tiles for both halves, all live (8 MiB fp8 total).
            # Separate tiles per 256-chunk keep the DMA->matmul dependency
            # per-chunk so the first tile's c-outer loop starts as soon as
            # chunk 0 lands (instead of waiting for the full 8 MiB).
            xh = [[xp.tile([128, 2, QTR], F8, tag=f"xc{c}", name=f"x{hf}c{c}")
                   for c in range(KC)] for hf in range(NQ)]

            def dma_x(hf, c):
                # half-1 (needed mid-iteration) on sync; half-0 rides the
                # otherwise-empty gpsimd queue so that in repeat mode the
                # NEXT iteration's half-0 prefetch fires right after this
                # iteration's half-1 triggers (~10% in) instead of behind
                # the whole W-stream -- the startup ramp then only exists
                # on the first iteration
                nc.sync.dma_start(xh[hf][c][:], x8_d[hf * KC + c])

            # first x chunk in quarters: the very first matmul needs only
            # 128 KiB of x + 64 KiB of W1 in flight, not 768 KiB
            for q in range(4):
                nc.gpsimd.dma_start(xh[0][0][:, :, 512 * q:512 * (q + 1)],
                                    x8_d[0][:, :, 512 * q:512 * (q + 1)])
            first = True
            for hf in range(NQ):
                for a in range(APC):
                    acc = [None] * NCH
                    # per-(head, chunk) epilogues (ones-reduce matmul +
                    # sigmoid + output DMA) are emitted one stage-1 group
                    # late so the in-order PE queue never waits on the
                    # DVE accumulation that produces their input.
                    pending = []

                    def tail(n, pt, a=a, hf=hf):
                        # runs per (m, n): gelu then DVE acc += ht * w2[m]
                        m = tail.m
                        ht = hp.tile([128, 512], BF, tag="ht", name="ht")
                        nc.scalar.activation(
                            ht[:], pt[:], AF.Gelu,
                            bias=b1t[:, a * MT + m:a * MT + m + 1],
                            scale=INV)
                        w2col = w2t[:, a * MT + m:a * MT + m + 1]
                        if m == 0:
                            acc_t = ap_.tile([128, 512], BF, tag=f"acc{n}",
                                             name="acc_t")
                            acc[n] = acc_t
                            nc.vector.tensor_scalar_mul(acc_t[:], ht[:], w2col)
                        else:
                            tmp = tp.tile([128, 512], BF, tag="tmp",
                                          name="tmp")
                            nc.vector.tensor_scalar_mul(tmp[:], ht[:], w2col)
                            nc.vector.tensor_tensor(acc[n][:], acc[n][:],
                                                    tmp[:], ALU.add)
                        if m == MT - 1:
                            def epilogue(n=n, a=a, hf=hf, acc_t=acc[n]):
                                psy = ps2.tile([1, 512], F32, tag="psy",
                                               name="psy")
                                nc.tensor.matmul(psy[:], ones[:], acc_t[:],
                                                 start=True, stop=True,
                                                 skip_group_check=True)
                                # sigmoid(z) = 0.5*tanh(z/2) + 0.5: tanh
                                # shares gelu's activation-table set, so no
                                # ACT_TABLE_LOAD ping-pong (HW-profiled at
                                # 1.5us per reload); b2p is pre-halved.
                                stg = sp.tile([1, 512], F32, tag="stg",
                                              name="stg")
                                nc.scalar.activation(
                                    stg[:], psy[:], AF.Tanh,
                                    bias=b2t[0:1, a:a + 1], scale=0.5)
                                stg2 = sp.tile([1, 512], F32, tag="stg2",
                                               name="stg2")
                                nc.vector.tensor_scalar(
                                    stg2[:], stg[:], 0.5, 0.5,
                                    ALU.mult, ALU.add)
                                nc.scalar.dma_start(
                                    y_d[a:a + 1,
                                        hf * QTR + n * 512:
                                        hf * QTR + (n + 1) * 512],
                                    stg2[:])
                            pending.append(epilogue)

                    if hf == NQ - 1 and a == APC - 1:
                        # Last head runs n-outer/m-inner: each batch-chunk's
                        # epilogue fires at 25/50/75/100% of this head's
                        # compute instead of all stacking after the final
                        # matmul (saves ~10us of end-of-kernel PE drain).
                        wl = []
                        for m in range(MT):
                            wt = wp.tile([128, KT, 128], F8, tag=f"wl{m}",
                                         name=f"wl{m}")
                            nc.sync.dma_start(wt[:], w1_d[a, m])
                            wl.append(wt)
                        for n in range(NCH):
                            for m in range(MT):
                                tail.m = m
                                pt = ps1.tile([128, 512], F32, tag="ps1")
                                for c in range(KC):
                                    nc.tensor.matmul(
                                        pt[:],
                                        wl[m][:, 2 * c:2 * c + 2, :],
                                        xh[hf][c][:, :,
                                                  n * 512:(n + 1) * 512],
                                        start=(c == 0), stop=(c == KC - 1),
                                        perf_mode=DR)
                                if pending:
                                    pending.pop(0)()
                                tail(n, pt)
                        while pending:
                            pending.pop(0)()
                        continue
                    w_next = None
                    for m in range(MT):
                        tail.m = m
                        if w_next is not None:
                            w = w_next
                            w_next = None
                        else:
                            w = wp.tile([128, KT, 128], F8, tag="w")
                            if first:
                                # ks-quads: the c-outer loop starts after
                                # ~192 KiB instead of the full 768 KiB of
                                # (W tile + x chunk)
                                for q in range(4):
                                    nc.sync.dma_start(
                                        w[:, 4 * q:4 * q + 4, :],
                                        w1_d[a, m, :, 4 * q:4 * q + 4, :])
                            else:
                                nc.sync.dma_start(w[:], w1_d[a, m])
                        if first and m == 0:
                            # pipeline the rest of half 0: two chunks in
                            # halves, then the m=1 W tile (so it lands well
                            # before the first tile's compute ends), then
                            # the remaining chunks in halves
                            def xhalves(c):
                                for i2 in range(2):
                                    nc.gpsimd.dma_start(
                                        xh[0][c][:, :,
                                                 1024 * i2:1024 * (i2 + 1)],
                                        x8_d[c][:, :,
                                                1024 * i2:1024 * (i2 + 1)])
                            for c in range(1, 3):
                                xhalves(c)
                            w_next = wp.tile([128, KT, 128], F8, tag="w",
                                             name="w_next")
                            nc.sync.dma_start(w_next[:], w1_d[a, 1])
                            for c in range(3, KC):
                                xhalves(c)
                        if hf == 0 and a == 0 and m == 4:
                            for c in range(KC):
                                dma_x(1, c)
                        # first three tiles run c-outermost: tiles after the
                        # first need ALL x chunks in pair mode, so they'd
                        # stall until the full 4 MiB half lands; c-outer
                        # rides the chunk-arrival wave instead.
                        kouter = first or (hf == 0 and a == 0 and m <= 4)
                        first = False
                        if kouter:
                            # c-outermost so each x chunk is consumed as it
                            # arrives; uses NCH psum banks at once.
                            pts = []
                            for n in range(NCH):
                                pt_n = ps1.tile([128, 512], F32, tag="ps1",
                                                name=f"pt{n}")
                                pts.append(pt_n)
                            for c in range(KC):
                                for n in range(NCH):
                                    nc.tensor.matmul(
                                        pts[n][:],
                                        w[:, 2 * c:2 * c + 2, :],
                                        xh[hf][c][:, :, n * 512:(n + 1) * 512],
                                        start=(c == 0), stop=(c == KC - 1),
                                        perf_mode=DR)
                            while pending:
                                pending.pop(0)()
                            for n in range(NCH):
                                tail(n, pts[n])
                        else:
                            # n-chunks in pairs, c outermost within the
                            # pair: consecutive matmuls share the stationary
                            # W1 slice (one LDWEIGHTS per 2 matmuls) and the
                            # pair boundary staggers PSUM-bank turnover.
                            for g in range(NCH // 2):
                                pts = [ps1.tile([128, 512], F32, tag="ps1",
                                                name=f"pt{g}{j}")
                                       for j in range(2)]
                                for c in range(KC):
                                    for j in range(2):
                                        n = 2 * g + j
                                        nc.tensor.matmul(
                                            pts[j][:],
                                            w[:, 2 * c:2 * c + 2, :],
                                            xh[hf][c][:, :,
                                                      n * 512:(n + 1) * 512],
                                            start=(c == 0), stop=(c == KC - 1),
                                            perf_mode=DR)
                                if pending:
                                    pending.pop(0)()
                                for j in range(2):
                                    tail(2 * g + j, pts[j])
                    while pending:
                        pending.pop(0)()

        if repeat and repeat > 1:
            with tc.For_i(0, repeat, 1):
                body()
        else:
            body()
    nc.compile()
    return nc


class _Runner:
    """jit-once PJRT runner for a prebuilt Bass program (8-core SPMD)."""

    def __init__(self, nc, n_cores):
        bass2jax.install_neuronx_cc_hook()
        self.nc = nc
        self.n_cores = n_cores
        in_names, out_names, out_avals, zero_outs = [], [], [], []
        for alloc in nc.m.functions[0].allocations:
            if not isinstance(alloc, mybir.MemoryLocationSet):
                continue
            name = alloc.memorylocations[0].name
            if alloc.kind == "ExternalInput":
                in_names.append(name)
            elif alloc.kind == "ExternalOutput":
                shape = tuple(alloc.tensor_shape)
                dtype = mybir.dt.np(alloc.dtype)
                out_names.append(name)
                out_avals.append(jax.core.ShapedArray(shape, dtype))
                zero_outs.append(np.zeros(shape, dtype))
        partition_name = (nc.partition_id_tensor.name
                          if nc.partition_id_tensor else None)
        if partition_name is not None and partition_name in in_names:
            in_names.remove(partition_name)
        self.in_names = in_names
        self.out_names = out_names
        self.zero_outs = zero_outs
        n_params = len(in_names)
        n_outs = len(out_avals)
        all_in_names = list(in_names) + list(out_names)
        if partition_name is not None:
            all_in_names.append(partition_name)
        donate = tuple(range(n_params, n_params + n_outs))

        def _body(*args):
            operands = list(args)
            if partition_name is not None:
                operands.append(bass2jax.partition_id_tensor())
            outs = bass2jax._bass_exec_p.bind(
                *operands,
                out_avals=tuple(out_avals),
                in_names=tuple(all_in_names),
                out_names=tuple(out_names),
                lowering_input_output_aliases=(),
                sim_require_finite=True,
                sim_require_nnan=True,
                nc=nc,
            )
            return tuple(outs)

        devices = jax.devices()[:n_cores]
        assert len(devices) == n_cores, f"need {n_cores} neuron cores"
        self.mesh = Mesh(np.asarray(devices), ("core",))
        in_specs = (PartitionSpec("core"),) * (n_params + n_outs)
        out_specs = (PartitionSpec("core"),) * n_outs
        self.fn = jax.jit(
            shard_map(_body, mesh=self.mesh, in_specs=in_specs,
                      out_specs=out_specs, check_rep=False),
            donate_argnums=donate, keep_unused=True,
        )
        self._dev_inputs = None

    def put_inputs(self, in_maps):
        sharding = NamedSharding(self.mesh, PartitionSpec("core"))
        self._dev_inputs = [
            jax.device_put(
                np.concatenate([np.asarray(m[name]) for m in in_maps], axis=0),
                sharding)
            for name in self.in_names
        ]

    def run(self):
        sharding = NamedSharding(self.mesh, PartitionSpec("core"))
        zouts = [jax.device_put(np.concatenate([z] * self.n_cores, axis=0),
                                sharding) for z in self.zero_outs]
        outs = self.fn(*self._dev_inputs, *zouts)
        jax.block_until_ready(outs)
        return outs

    def run_np(self):
        outs = self.run()
        res = []
        for c in range(self.n_cores):
            d = {}
            for i, name in enumerate(self.out_names):
                full = np.asarray(outs[i])
                per = full.shape[0] // self.n_cores
                d[name] = full[c * per:(c + 1) * per]
            res.append(d)
        return res


_CACHE = {}


def _get_runner(repeat=0):
    if repeat not in _CACHE:
        _CACHE[repeat] = _Runner(build_program(repeat), NCORES)
    return _CACHE[repeat]


def _q8(a, scale):
    return np.clip(np.asarray(a, np.float32) * scale,
                   -240.0, 240.0).astype(E4NP)


def make_in_maps(x, W1, b1, W2, b2):
    x = np.asarray(x, dtype=np.float32)
    W1 = np.asarray(W1, dtype=np.float32)
    b1 = np.asarray(b1, dtype=np.float32)
    W2 = np.asarray(W2, dtype=np.float32)
    b2 = np.asarray(b2, dtype=np.float32)
    # x8[(hf*KC+c), p, i*QTR+n] = fp8(SX * x[hf*QTR+n, c*256+i*128+p]):
    # per-chunk contiguous 512 KiB blocks matching the [128, 2, QTR] tiles
    xq = _q8(x, SX).T
    x8 = np.ascontiguousarray(
        xq.reshape(KC, 2, 128, NQ, QTR).transpose(3, 0, 2, 1, 4)
        .reshape(NQ * KC, 128, 2, QTR))
    # W1p[a, m, p, ks, c] = fp8(SW * W1[a, ks*128+p, m*128+c]) (per-(a,m)
    # contiguous 256 KiB block; DoubleRow pairs are ks slots (2c, 2c+1))
    W1p = np.ascontiguousarray(
        _q8(W1, SW).reshape(A, KT, 128, MT, 128).transpose(0, 3, 2, 1, 4)
        .reshape(A, MT, 128, KT, 128))
    b1p = np.ascontiguousarray(b1.reshape(A, MT, 128).transpose(0, 2, 1))
    W2p = np.ascontiguousarray(W2.reshape(A, MT, 128).transpose(0, 2, 1))
    b2p = np.ascontiguousarray(0.5 * b2.reshape(1, A))  # tanh-form sigmoid
    in_maps = []
    for c in range(NCORES):
        s = slice(c * APC, (c + 1) * APC)
        in_maps.append({"x8": x8, "w1p": W1p[s], "b1p": b1p[s],
                        "w2p": W2p[s], "b2p": b2p[:, s]})
    return in_maps


def kernel(x, W1, b1, W2, b2):
    in_maps = make_in_maps(x, W1, b1, W2, b2)
    r = _get_runner(0)
    r.put_inputs(in_maps)
    outs = r.run_np()
    y = np.concatenate([outs[c]["y"] for c in range(NCORES)], axis=0)
    return np.ascontiguousarray(y.T).astype(np.float32)
